# revision 3
# baseline (speedup 1.0000x reference)
"""Causal self-attention (B=2, T=2048, C=1024, H=16) on 8 trn2 NeuronCores.

Sharding (Megatron-style): core p owns heads {2p, 2p+1}; computes Q/K/V^T
for its heads from full x, causal attention (streaming softmax, denominator
via ones-column in V), then a head-split AllToAll redistributes outputs so
core p holds all 1024 channels for token blocks [128p, 128p+128) of each
1024-token half; output projection is local per 128-token tile.

v2 schedule (vs baseline):
  - all 8 x chunks prefetched up front, each split across the sync+scalar
    HWDGE queues; per-chunk persistent tiles let attention(qc) start as
    soon as qkv(qc) lands.
  - qkv processed in chunk-pairs with et-ct-outer loops so consecutive
    matmuls share a stationary tile (fewer LDWEIGHTS columns on the PE bus).
  - S^T and exp trimmed to causal columns on diagonal k-tiles.
  - projection bias via vector add against a DMA-broadcast bias tile
    (no bias matmuls).
  - AllToAll split per head-half (2 collectives of 131KB per batch-half):
    staging fires per q-chunk, and the tail's last collective overlaps
    proj of the previous part.
"""

import numpy as np

B, T, C, H, D = 2, 2048, 1024, 16, 64
NCORES = 8
HL = H // NCORES        # heads per core = 2
TOK = B * T
TSL = TOK // NCORES     # 512 output tokens per core
SL = 256                # per-batch token slice per core
P = 128
CT = C // P             # 8 contraction tiles
NQC = T // 512          # 4 q-chunks per batch
SCALE = D ** -0.5

_CACHE = {}


def _build_nc():
    import concourse.bass as bass
    import concourse.mybir as mybir
    from concourse import bacc
    from concourse.tile import TileContext

    f32 = mybir.dt.float32
    bf16 = mybir.dt.bfloat16
    AF = mybir.ActivationFunctionType
    ALU = mybir.AluOpType

    nc = bacc.Bacc(
        "TRN2", target_bir_lowering=False, debug=False, num_devices=NCORES
    )

    xT = nc.dram_tensor("xT", [C, TOK], bf16, kind="ExternalInput")
    wqkvT = nc.dram_tensor("wqkvT", [C, 3 * P], bf16, kind="ExternalInput")
    bqkv = nc.dram_tensor("bqkv", [3 * P], f32, kind="ExternalInput")
    wpA = nc.dram_tensor("wpA", [4 * P, C], bf16, kind="ExternalInput")
    wpB = nc.dram_tensor("wpB", [4 * P, C], bf16, kind="ExternalInput")
    bp = nc.dram_tensor("bp", [C], f32, kind="ExternalInput")
    tri = nc.dram_tensor("tri", [P, P], bf16, kind="ExternalInput")
    ident = nc.dram_tensor("ident", [P, P], bf16, kind="ExternalInput")
    y = nc.dram_tensor("y", [TSL, C], f32, kind="ExternalOutput")

    with TileContext(nc, num_cores=NCORES) as tc:
        from contextlib import ExitStack

        with ExitStack() as ctx:
            const = ctx.enter_context(tc.tile_pool(name="const", bufs=1))
            persist = ctx.enter_context(tc.tile_pool(name="persist", bufs=1))
            dram = ctx.enter_context(tc.tile_pool(name="dram", bufs=1, space="DRAM"))

            # ---- constants (gpsimd queue: small, early)
            tri_sb = const.tile([P, P], bf16)
            id_sb = const.tile([P, P], bf16)
            bq_sb = const.tile([P, 3], f32)
            bp_bc = const.tile([P, C], f32)
            w_sb = const.tile([P, CT, 3 * P], bf16)
            wpA_sb = const.tile([P, 4, C], bf16)
            wpB_sb = const.tile([P, 4, C], bf16)
            nc.gpsimd.dma_start(tri_sb[:], tri[:])
            nc.gpsimd.dma_start(id_sb[:], ident[:])
            nc.gpsimd.dma_start(bq_sb[:], bqkv.rearrange("(et p) -> p et", p=P))

            # ---- x chunks: all prefetched, halves split across queues
            xc = [[persist.tile([P, CT, 512], bf16, name=f"x{b}{c}")
                   for c in range(NQC)] for b in range(B)]
            # sync queue: qkv weights first, then even ct-halves in use order
            nc.sync.dma_start(w_sb[:], wqkvT.rearrange("(ct p) e -> p ct e", p=P))
            for b in range(B):
                for c in range(NQC):
                    t0 = b * T + c * 512
                    src = xT[:, t0:t0 + 512].rearrange("(ct p) t -> p ct t", p=P)
                    nc.sync.dma_start(xc[b][c][:, 0:4, :], src[:, 0:4, :])
                    nc.scalar.dma_start(xc[b][c][:, 4:8, :], src[:, 4:8, :])
            # late-need consts
            nc.gpsimd.dma_start(
                bp_bc[:],
                bp.rearrange("(o c) -> o c", o=1).to_broadcast((P, C)),
            )
            nc.gpsimd.dma_start(wpA_sb[:], wpA.rearrange("(p i) f -> p i f", p=P))
            nc.gpsimd.dma_start(wpB_sb[:], wpB.rearrange("(p i) f -> p i f", p=P))

            # ---- per-chunk persistent activations
            qTc = [[persist.tile([P, 512], bf16, name=f"q{b}{c}")
                    for c in range(NQC)] for b in range(B)]
            kTc = [[persist.tile([P, 512], bf16, name=f"k{b}{c}")
                    for c in range(NQC)] for b in range(B)]
            # V with ones column: [128 tok, kt, 2*65]
            vaug = [[persist.tile([P, 4, 2 * 65], bf16, name=f"va{b}{c}")
                     for c in range(NQC)] for b in range(B)]
            # normalized A^T (+denominator row) per (b, qc, h)
            anf = [[[persist.tile([65, 512], f32, name=f"an{b}{c}{h}")
                     for h in range(HL)] for c in range(NQC)] for b in range(B)]
            # a2a buffers per (b, part, head-half)
            a2a_in = [[[dram.tile([NCORES * 64, P], bf16,
                                  name=f"ai{b}{pr}{h}")
                        for h in range(HL)] for pr in range(2)] for b in range(B)]
            a2a_out = [[[dram.tile([NCORES * 64, P], bf16,
                                   name=f"ao{b}{pr}{h}")
                         for h in range(HL)] for pr in range(2)] for b in range(B)]
            rdrs = {}

            pools = [
                tc.tile_pool(name="sps", bufs=2, space="PSUM"),   # 2x2 banks
                tc.tile_pool(name="ops", bufs=2, space="PSUM"),   # 2x1 bank
                tc.tile_pool(name="mm", bufs=2, space="PSUM"),    # 2x1 bank
                tc.tile_pool(name="pT", bufs=2),
                tc.tile_pool(name="vt", bufs=2),
                tc.tile_pool(name="rp", bufs=2),
                tc.tile_pool(name="rb", bufs=2),
                tc.tile_pool(name="afull", bufs=4),
                tc.tile_pool(name="ysb", bufs=2),
            ]
            sps, ops, mm, ppool, vtpool, rppool, rbpool, apool, ypool = (
                ctx.enter_context(p) for p in pools)

            def qkv_pair(b, cp):
                """qkv^T for chunks (2cp, 2cp+1); et-ct-outer for ldw reuse."""
                c0 = 2 * cp
                vts = {}
                for et in range(3):
                    ps = sps.tile([P, 2, 512], f32, tag="s")
                    for ct in range(CT):
                        for ci in range(2):
                            nc.tensor.matmul(
                                ps[:, ci, :],
                                lhsT=w_sb[:, ct, et * P:(et + 1) * P],
                                rhs=xc[b][c0 + ci][:, ct, :],
                                start=(ct == 0),
                                stop=(ct == CT - 1),
                            )
                    for ci in range(2):
                        c = c0 + ci
                        if et == 0:
                            dst = qTc[b][c][:]
                        elif et == 1:
                            dst = kTc[b][c][:]
                        else:
                            dst = vtpool.tile([P, 512], bf16, tag="vt")
                            vts[c] = dst
                        nc.vector.tensor_scalar_add(
                            dst if et == 2 else dst,
                            ps[:, ci, :],
                            bq_sb[:, et:et + 1],
                        )
                # V^T -> V (PE transpose) + vaug fill
                for ci in range(2):
                    c = c0 + ci
                    nc.vector.memset(vaug[b][c][:, :, 64::65], 1.0)
                    for kt4 in range(4):
                        tp = mm.tile([P, P], bf16, tag="mm")
                        nc.tensor.transpose(
                            tp[:],
                            vts[c][:, kt4 * P:(kt4 + 1) * P],
                            id_sb[:],
                        )
                        nc.vector.tensor_copy(
                            vaug[b][c][:, kt4, 0:2 * 65]
                            .rearrange("p (h e) -> p h e", h=2)[:, :, 0:64],
                            tp.rearrange("p (h e) -> p h e", h=2),
                        )

            def attention_qc(b, qc):
                q0 = qc * 512
                nk = 4 * qc + 4
                o_t = [ops.tile([65, 512], f32, tag="o", name=f"ot{hh}")
                       for hh in range(HL)]
                for ki in range(nk):
                    off = ki * P - q0
                    lo = max(0, off)
                    sp = sps.tile([P, HL, 512], f32, tag="s")
                    for h in range(HL):
                        hp = slice(64 * h, 64 * h + 64)
                        nc.tensor.matmul(
                            sp[:, h, lo:512],
                            lhsT=kTc[b][ki // 4][hp, (ki % 4) * P:(ki % 4 + 1) * P],
                            rhs=qTc[b][qc][hp, lo:512],
                            start=True,
                            stop=True,
                        )
                    pt = ppool.tile([P, HL, 512], bf16, tag="p")
                    nc.scalar.activation(
                        pt[:, :, lo:512], sp[:, :, lo:512], AF.Exp, scale=SCALE,
                    )
                    if off >= 0:
                        for h in range(HL):
                            nc.vector.tensor_tensor(
                                pt[:, h, off:off + P],
                                pt[:, h, off:off + P],
                                tri_sb[:],
                                ALU.mult,
                            )
                    for h in range(HL):
                        nc.tensor.matmul(
                            o_t[h][:, lo:512],
                            lhsT=vaug[b][ki // 4][:, ki % 4, h * 65:h * 65 + 65],
                            rhs=pt[:, h, lo:512],
                            start=(ki == 0),
                            stop=(ki == nk - 1),
                        )
                # epilogue: stash, normalize, stage into a2a buffers
                part, j0 = qc // 2, 4 * (qc % 2)
                for h in range(HL):
                    nc.vector.tensor_copy(anf[b][qc][h][:], o_t[h][:])
                for h in range(HL):
                    at = anf[b][qc][h]
                    dpk = rppool.tile([8, 64], f32, tag="dpk")
                    rpk = rppool.tile([8, 64], f32, tag="rpk")
                    rsc = rppool.tile([8, 64], f32, tag="rsc")
                    nc.sync.dma_start(dpk[:], at[64:65, :])
                    nc.vector.reciprocal_approx_accurate(rpk[:], dpk[:], rsc[:])
                    rdr = dram.tile([1, 512], f32, name=f"rd{b}{qc}{h}")
                    rdrs[(b, qc, h)] = rdr
                    nc.sync.dma_start(
                        rdr.rearrange("o (rr f) -> (o rr) f", f=64), rpk[:],
                    )
                    rb = rbpool.tile([64, 512], f32, tag="rb")
                    nc.sync.dma_start(rb[:], rdr.to_broadcast((64, 512)))
                    nc.vector.tensor_tensor(
                        at[0:64, :], at[0:64, :], rb[:], ALU.mult,
                    )
                    # stage this q-chunk's half of the (b, part, h) payload
                    nc.gpsimd.dma_start(
                        a2a_in[b][part][h]
                        .rearrange("(j e) t -> e j t", j=NCORES)[:, j0:j0 + 4, :],
                        at[0:64, :].rearrange("e (j t) -> e j t", j=4),
                    )

            def a2a_fire(b, part):
                from concourse import mybir as mb
                for h in range(HL):
                    nc.gpsimd.collective_compute(
                        "AllToAll",
                        mb.AluOpType.bypass,
                        replica_groups=[list(range(NCORES))],
                        ins=[a2a_in[b][part][h].opt()],
                        outs=[a2a_out[b][part][h].opt()],
                    )

            def proj_part(b, part):
                afs = []
                for h in range(HL):
                    af = apool.tile([P, 4, P], bf16, tag="af")
                    nc.scalar.dma_start(
                        af[:],
                        a2a_out[b][part][h]
                        .rearrange("(i4 i2 e) t -> (i2 e) i4 t", i4=4, i2=2),
                    )
                    afs.append(af)
                pss = []
                for fc in range(2):
                    ps = mm.tile([P, 512], f32, tag="mm")
                    pss.append(ps)
                    for i4 in range(4):
                        nc.tensor.matmul(
                            ps[:],
                            lhsT=afs[0][:, i4, :],
                            rhs=wpA_sb[:, i4, fc * 512:(fc + 1) * 512],
                            start=(i4 == 0),
                            stop=False,
                        )
                for fc in range(2):
                    ps = pss[fc]
                    for i4 in range(4):
                        nc.tensor.matmul(
                            ps[:],
                            lhsT=afs[1][:, i4, :],
                            rhs=wpB_sb[:, i4, fc * 512:(fc + 1) * 512],
                            start=False,
                            stop=(i4 == 3),
                        )
                    ysb = ypool.tile([P, 512], f32, tag="ysb")
                    nc.vector.tensor_tensor(
                        ysb[:], ps[:], bp_bc[:, fc * 512:(fc + 1) * 512],
                        ALU.add,
                    )
                    r0 = b * SL + part * P
                    nc.sync.dma_start(
                        y[r0:r0 + P, fc * 512:(fc + 1) * 512], ysb[:],
                    )

            # ---------------- schedule ----------------
            qkv_pair(0, 0)
            attention_qc(0, 0)
            attention_qc(0, 1)
            a2a_fire(0, 0)
            qkv_pair(0, 1)
            attention_qc(0, 2)
            attention_qc(0, 3)
            a2a_fire(0, 1)
            qkv_pair(1, 0)
            proj_part(0, 0)
            attention_qc(1, 0)
            attention_qc(1, 1)
            a2a_fire(1, 0)
            qkv_pair(1, 1)
            proj_part(0, 1)
            attention_qc(1, 2)
            attention_qc(1, 3)
            proj_part(1, 0)
            a2a_fire(1, 1)
            proj_part(1, 1)
    nc.compile()
    return nc


def _prep_inputs(x, W_qkv, b_qkv, W_proj, b_proj):
    x = np.asarray(x, dtype=np.float32)
    W_qkv = np.asarray(W_qkv, dtype=np.float32)
    b_qkv = np.asarray(b_qkv, dtype=np.float32)
    W_proj = np.asarray(W_proj, dtype=np.float32)
    b_proj = np.asarray(b_proj, dtype=np.float32)

    import ml_dtypes
    bf = ml_dtypes.bfloat16
    xT = np.ascontiguousarray(x.reshape(TOK, C).T).astype(bf)
    wpT = np.ascontiguousarray(W_proj.T)          # [in-ch, f]
    tri = np.triu(np.ones((P, P), dtype=np.float32)).astype(bf)
    ident = np.eye(P, dtype=np.float32).astype(bf)

    # head-split proj weights: rows [(i2 e), i4] -> ch = 128*(2*i4+i2)+64*h+e
    wpX = []
    for h in range(HL):
        idx = np.empty((P, 4), dtype=np.int64)
        for i2 in range(2):
            for e in range(64):
                for i4 in range(4):
                    idx[i2 * 64 + e, i4] = 128 * (2 * i4 + i2) + 64 * h + e
        wpX.append(np.ascontiguousarray(wpT[idx.reshape(-1)]).astype(bf))

    in_maps = []
    for p in range(NCORES):
        rows = np.r_[128 * p:128 * p + 128,
                     C + 128 * p:C + 128 * p + 128,
                     2 * C + 128 * p:2 * C + 128 * p + 128]
        wslice = W_qkv[rows]
        bslice = np.ascontiguousarray(b_qkv[rows])
        in_maps.append({
            "xT": xT,
            "wqkvT": np.ascontiguousarray(wslice.T).astype(bf),
            "bqkv": bslice,
            "wpA": wpX[0],
            "wpB": wpX[1],
            "bp": b_proj,
            "tri": tri,
            "ident": ident,
        })
    return in_maps


def kernel(x, W_qkv, b_qkv, W_proj, b_proj, _trace=False):
    from concourse import bass_utils

    if "nc" not in _CACHE:
        _CACHE["nc"] = _build_nc()
    nc = _CACHE["nc"]
    in_maps = _prep_inputs(x, W_qkv, b_qkv, W_proj, b_proj)
    res = bass_utils.run_bass_kernel_spmd(
        nc, in_maps, core_ids=list(range(NCORES)), trace=_trace,
    )
    _CACHE["last_result"] = res
    yfull = np.empty((B, T, C), dtype=np.float32)
    for p, rmap in enumerate(res.results):
        yp = rmap["y"]
        for b in range(B):
            for part in range(2):
                g0 = part * 1024 + 128 * p
                r0 = b * SL + part * P
                yfull[b, g0:g0 + P] = yp[r0:r0 + P]
    return yfull


# revision 10
# speedup vs baseline: 1.1585x; 1.1585x over previous
"""Causal self-attention (B=2, T=2048, C=1024, H=16) on 8 trn2 NeuronCores.

Sharding (Megatron-style): core p owns heads {2p, 2p+1}; computes Q/K/V^T
for its heads from full x, causal attention (streaming softmax, denominator
via ones-column in V), then a head-split AllToAll redistributes outputs so
core p holds all 1024 channels for token blocks [128p, 128p+128) of each
1024-token half; output projection is local per 128-token tile.

v2 schedule (vs baseline):
  - all 8 x chunks prefetched up front, each split across the sync+scalar
    HWDGE queues; per-chunk persistent tiles let attention(qc) start as
    soon as qkv(qc) lands.
  - qkv processed in chunk-pairs with et-ct-outer loops so consecutive
    matmuls share a stationary tile (fewer LDWEIGHTS columns on the PE bus).
  - S^T and exp trimmed to causal columns on diagonal k-tiles.
  - projection bias via vector add against a DMA-broadcast bias tile
    (no bias matmuls).
  - AllToAll split per head-half (2 collectives of 131KB per batch-half):
    staging fires per q-chunk, and the tail's last collective overlaps
    proj of the previous part.
"""

import numpy as np

B, T, C, H, D = 2, 2048, 1024, 16, 64
NCORES = 8
HL = H // NCORES        # heads per core = 2
TOK = B * T
TSL = TOK // NCORES     # 512 output tokens per core
SL = 256                # per-batch token slice per core
P = 128
CT = C // P             # 8 contraction tiles
NQC = T // 512          # 4 q-chunks per batch
SCALE = D ** -0.5

_CACHE = {}


def _build_nc():
    import concourse.bass as bass
    import concourse.mybir as mybir
    from concourse import bacc
    from concourse.tile import TileContext

    f32 = mybir.dt.float32
    bf16 = mybir.dt.bfloat16
    AF = mybir.ActivationFunctionType
    ALU = mybir.AluOpType

    nc = bacc.Bacc(
        "TRN2", target_bir_lowering=False, debug=False, num_devices=NCORES
    )

    xT = nc.dram_tensor("xT", [C, TOK], bf16, kind="ExternalInput")
    wqkvT = nc.dram_tensor("wqkvT", [C, 3 * P], bf16, kind="ExternalInput")
    bqkv = nc.dram_tensor("bqkv", [3 * P], f32, kind="ExternalInput")
    wpT = nc.dram_tensor("wpT", [C, C], bf16, kind="ExternalInput")
    bp = nc.dram_tensor("bp", [C], f32, kind="ExternalInput")
    tri = nc.dram_tensor("tri", [P, P], bf16, kind="ExternalInput")
    ident = nc.dram_tensor("ident", [P, P], bf16, kind="ExternalInput")
    y = nc.dram_tensor("y", [TSL, C], f32, kind="ExternalOutput")

    with TileContext(nc, num_cores=NCORES) as tc:
        from contextlib import ExitStack

        with ExitStack() as ctx:
            const = ctx.enter_context(tc.tile_pool(name="const", bufs=1))
            persist = ctx.enter_context(tc.tile_pool(name="persist", bufs=1))
            dram = ctx.enter_context(tc.tile_pool(name="dram", bufs=1, space="DRAM"))

            # ---- constants + qkv weights (gpsimd queue: early, off the
            # x-load queues)
            tri_sb = const.tile([P, P], bf16)
            id_sb = const.tile([P, P], bf16)
            bq_sb = const.tile([P, 3], f32)
            bp_bc = const.tile([P, C], f32)
            w_sb = const.tile([P, CT, 3 * P], bf16)
            wp_sb = const.tile([P, CT, C], bf16)
            nc.gpsimd.dma_start(tri_sb[:], tri[:])
            nc.gpsimd.dma_start(id_sb[:], ident[:])
            nc.gpsimd.dma_start(bq_sb[:], bqkv.rearrange("(et p) -> p et", p=P))
            nc.gpsimd.dma_start(
                w_sb[:], wqkvT.rearrange("(ct p) e -> p ct e", p=P))

            # warmup collective: absorbs the one-time cc rendezvous cost
            # while the PE is still waiting on the first x chunk
            cc_wu_in = dram.tile([NCORES, 64], bf16, name="ccwi")
            cc_wu_out = dram.tile([NCORES, 64], bf16, name="ccwo")
            nc.gpsimd.collective_compute(
                "AllToAll",
                mybir.AluOpType.bypass,
                replica_groups=[list(range(NCORES))],
                ins=[cc_wu_in.opt()],
                outs=[cc_wu_out.opt()],
            )

            # ---- x chunks: all prefetched, halves split across queues
            xc = [[persist.tile([P, CT, 512], bf16, name=f"x{b}{c}")
                   for c in range(NQC)] for b in range(B)]
            for b in range(B):
                for c in range(NQC):
                    t0 = b * T + c * 512
                    src = xT[:, t0:t0 + 512].rearrange("(ct p) t -> p ct t", p=P)
                    nc.sync.dma_start(xc[b][c][:, 0:4, :], src[:, 0:4, :])
                    nc.scalar.dma_start(xc[b][c][:, 4:8, :], src[:, 4:8, :])
            # late-need consts
            nc.gpsimd.dma_start(
                bp_bc[:],
                bp.rearrange("(o c) -> o c", o=1).to_broadcast((P, C)),
            )
            nc.gpsimd.dma_start(
                wp_sb[:], wpT.rearrange("(ct p) f -> p ct f", p=P))

            # ---- per-chunk persistent activations
            qTc = [[persist.tile([P, 512], bf16, name=f"q{b}{c}")
                    for c in range(NQC)] for b in range(B)]
            kTc = [[persist.tile([P, 512], bf16, name=f"k{b}{c}")
                    for c in range(NQC)] for b in range(B)]
            # V with ones column: [128 tok, kt, 2*65]
            vaug = [[persist.tile([P, 4, 2 * 65], bf16, name=f"va{b}{c}")
                     for c in range(NQC)] for b in range(B)]
            # normalized A^T (+denominator row) per (b, qc, h)
            anf = [[[persist.tile([65, 512], f32, name=f"an{b}{c}{h}")
                     for h in range(HL)] for c in range(NQC)] for b in range(B)]
            # a2a buffers per (b, part): rows (j, ee) = (dest core, my 128 ch)
            a2a_in = [[dram.tile([NCORES * P, P], bf16, name=f"ai{b}{pr}")
                       for pr in range(2)] for b in range(B)]
            a2a_out = [[dram.tile([NCORES * P, P], bf16, name=f"ao{b}{pr}")
                        for pr in range(2)] for b in range(B)]

            pools = [
                tc.tile_pool(name="sps", bufs=2, space="PSUM"),   # 2x2 banks
                tc.tile_pool(name="ops", bufs=2, space="PSUM"),   # 2x1 bank
                tc.tile_pool(name="mm", bufs=2, space="PSUM"),    # 2x1 bank
                tc.tile_pool(name="pT", bufs=2),
                tc.tile_pool(name="vt", bufs=2),
                tc.tile_pool(name="rp", bufs=2),
                tc.tile_pool(name="rb", bufs=2),
                tc.tile_pool(name="afull", bufs=2),
                tc.tile_pool(name="ysb", bufs=2),
            ]
            sps, ops, mm, ppool, vtpool, rppool, rbpool, apool, ypool = (
                ctx.enter_context(p) for p in pools)

            def qkv_pair(b, cp):
                """qkv^T for chunks (2cp, 2cp+1); et-ct-outer for ldw reuse."""
                c0 = 2 * cp
                vts = {}
                for et in range(3):
                    ps = sps.tile([P, 2, 512], f32, tag="s")
                    for ct in range(CT):
                        for ci in range(2):
                            nc.tensor.matmul(
                                ps[:, ci, :],
                                lhsT=w_sb[:, ct, et * P:(et + 1) * P],
                                rhs=xc[b][c0 + ci][:, ct, :],
                                start=(ct == 0),
                                stop=(ct == CT - 1),
                            )
                    for ci in range(2):
                        c = c0 + ci
                        if et == 0:
                            dst = qTc[b][c][:]
                        elif et == 1:
                            dst = kTc[b][c][:]
                        else:
                            dst = vtpool.tile([P, 512], bf16, tag="vt")
                            vts[c] = dst
                        nc.vector.tensor_scalar_add(
                            dst if et == 2 else dst,
                            ps[:, ci, :],
                            bq_sb[:, et:et + 1],
                        )
                # V^T -> V (PE transpose) + vaug fill
                for ci in range(2):
                    c = c0 + ci
                    nc.vector.memset(vaug[b][c][:, :, 64::65], 1.0)
                    for kt4 in range(4):
                        tp = mm.tile([P, P], bf16, tag="mm")
                        nc.tensor.transpose(
                            tp[:],
                            vts[c][:, kt4 * P:(kt4 + 1) * P],
                            id_sb[:],
                        )
                        nc.vector.tensor_copy(
                            vaug[b][c][:, kt4, 0:2 * 65]
                            .rearrange("p (h e) -> p h e", h=2)[:, :, 0:64],
                            tp.rearrange("p (h e) -> p h e", h=2),
                        )

            def attention_qc(b, qc):
                q0 = qc * 512
                nk = 4 * qc + 4
                o_t = [ops.tile([65, 512], f32, tag="o", name=f"ot{hh}")
                       for hh in range(HL)]
                for ki in range(nk):
                    off = ki * P - q0
                    lo = max(0, off)
                    sp = sps.tile([P, HL, 512], f32, tag="s")
                    for h in range(HL):
                        hp = slice(64 * h, 64 * h + 64)
                        nc.tensor.matmul(
                            sp[:, h, lo:512],
                            lhsT=kTc[b][ki // 4][hp, (ki % 4) * P:(ki % 4 + 1) * P],
                            rhs=qTc[b][qc][hp, lo:512],
                            start=True,
                            stop=True,
                        )
                    pt = ppool.tile([P, HL, 512], bf16, tag="p")
                    nc.scalar.activation(
                        pt[:, :, lo:512], sp[:, :, lo:512], AF.Exp, scale=SCALE,
                    )
                    if off >= 0:
                        for h in range(HL):
                            nc.vector.tensor_tensor(
                                pt[:, h, off:off + P],
                                pt[:, h, off:off + P],
                                tri_sb[:],
                                ALU.mult,
                            )
                    for h in range(HL):
                        nc.tensor.matmul(
                            o_t[h][:, lo:512],
                            lhsT=vaug[b][ki // 4][:, ki % 4, h * 65:h * 65 + 65],
                            rhs=pt[:, h, lo:512],
                            start=(ki == 0),
                            stop=(ki == nk - 1),
                        )
                # epilogue: stash, normalize, stage into a2a buffers
                part, j0 = qc // 2, 4 * (qc % 2)
                for h in range(HL):
                    nc.vector.tensor_copy(anf[b][qc][h][:], o_t[h][:])
                for h in range(HL):
                    at = anf[b][qc][h]
                    dpk = rppool.tile([8, 64], f32, tag="dpk")
                    rpk = rppool.tile([8, 64], f32, tag="rpk")
                    rsc = rppool.tile([8, 64], f32, tag="rsc")
                    nc.sync.dma_start(dpk[:], at[64:65, :])
                    nc.vector.reciprocal_approx_accurate(rpk[:], dpk[:], rsc[:])
                    rdr = dram.tile([1, 512], f32, name=f"rd{b}{qc}{h}")
                    nc.sync.dma_start(
                        rdr.rearrange("o (rr f) -> (o rr) f", f=64), rpk[:],
                    )
                    rb = rbpool.tile([64, 512], f32, tag="rb")
                    nc.sync.dma_start(rb[:], rdr.to_broadcast((64, 512)))
                    nc.vector.tensor_tensor(
                        at[0:64, :], at[0:64, :], rb[:], ALU.mult,
                    )
                    # stage this q-chunk's quarter of the (b, part) payload
                    nc.gpsimd.dma_start(
                        a2a_in[b][part]
                        .rearrange("(j ee) t -> ee j t", j=NCORES)
                        [64 * h:64 * h + 64, j0:j0 + 4, :],
                        at[0:64, :].rearrange("e (j t) -> e j t", j=4),
                    )

            def a2a_fire(b, part):
                nc.gpsimd.collective_compute(
                    "AllToAll",
                    mybir.AluOpType.bypass,
                    replica_groups=[list(range(NCORES))],
                    ins=[a2a_in[b][part].opt()],
                    outs=[a2a_out[b][part].opt()],
                )

            def proj_part(b, part):
                af = apool.tile([P, NCORES, P], bf16, tag="af")
                nc.scalar.dma_start(
                    af[:],
                    a2a_out[b][part].rearrange("(i e) t -> e i t", i=NCORES),
                )
                for fc in range(2):
                    ps = mm.tile([P, 512], f32, tag="mm")
                    for i in range(NCORES):
                        nc.tensor.matmul(
                            ps[:],
                            lhsT=af[:, i, :],
                            rhs=wp_sb[:, i, fc * 512:(fc + 1) * 512],
                            start=(i == 0),
                            stop=(i == NCORES - 1),
                        )
                    ysb = ypool.tile([P, 512], f32, tag="ysb")
                    nc.vector.tensor_tensor(
                        ysb[:], ps[:], bp_bc[:, fc * 512:(fc + 1) * 512],
                        ALU.add,
                    )
                    r0 = b * SL + part * P
                    nc.sync.dma_start(
                        y[r0:r0 + P, fc * 512:(fc + 1) * 512], ysb[:],
                    )

            # ---------------- schedule ----------------
            qkv_pair(0, 0)
            attention_qc(0, 0)
            attention_qc(0, 1)
            a2a_fire(0, 0)
            qkv_pair(0, 1)
            attention_qc(0, 2)
            attention_qc(0, 3)
            a2a_fire(0, 1)
            qkv_pair(1, 0)
            proj_part(0, 0)
            attention_qc(1, 0)
            attention_qc(1, 1)
            a2a_fire(1, 0)
            qkv_pair(1, 1)
            proj_part(0, 1)
            attention_qc(1, 2)
            attention_qc(1, 3)
            proj_part(1, 0)
            a2a_fire(1, 1)
            proj_part(1, 1)
    nc.compile()
    return nc


def _prep_inputs(x, W_qkv, b_qkv, W_proj, b_proj):
    x = np.asarray(x, dtype=np.float32)
    W_qkv = np.asarray(W_qkv, dtype=np.float32)
    b_qkv = np.asarray(b_qkv, dtype=np.float32)
    W_proj = np.asarray(W_proj, dtype=np.float32)
    b_proj = np.asarray(b_proj, dtype=np.float32)

    import ml_dtypes
    bf = ml_dtypes.bfloat16
    xT = np.ascontiguousarray(x.reshape(TOK, C).T).astype(bf)
    wpT = np.ascontiguousarray(W_proj.T).astype(bf)
    tri = np.triu(np.ones((P, P), dtype=np.float32)).astype(bf)
    ident = np.eye(P, dtype=np.float32).astype(bf)

    in_maps = []
    for p in range(NCORES):
        rows = np.r_[128 * p:128 * p + 128,
                     C + 128 * p:C + 128 * p + 128,
                     2 * C + 128 * p:2 * C + 128 * p + 128]
        wslice = W_qkv[rows]
        bslice = np.ascontiguousarray(b_qkv[rows])
        in_maps.append({
            "xT": xT,
            "wqkvT": np.ascontiguousarray(wslice.T).astype(bf),
            "bqkv": bslice,
            "wpT": wpT,
            "bp": b_proj,
            "tri": tri,
            "ident": ident,
        })
    return in_maps


def kernel(x, W_qkv, b_qkv, W_proj, b_proj, _trace=False):
    from concourse import bass_utils

    if "nc" not in _CACHE:
        _CACHE["nc"] = _build_nc()
    nc = _CACHE["nc"]
    in_maps = _prep_inputs(x, W_qkv, b_qkv, W_proj, b_proj)
    res = bass_utils.run_bass_kernel_spmd(
        nc, in_maps, core_ids=list(range(NCORES)), trace=_trace,
    )
    _CACHE["last_result"] = res
    yfull = np.empty((B, T, C), dtype=np.float32)
    for p, rmap in enumerate(res.results):
        yp = rmap["y"]
        for b in range(B):
            for part in range(2):
                g0 = part * 1024 + 128 * p
                r0 = b * SL + part * P
                yfull[b, g0:g0 + P] = yp[r0:r0 + P]
    return yfull


# revision 20
# speedup vs baseline: 1.1870x; 1.0246x over previous
"""Causal self-attention (B=2, T=2048, C=1024, H=16) on 8 trn2 NeuronCores.

Sharding (Megatron-style): core p owns heads {2p, 2p+1}; computes Q/K/V^T
for its heads from full x, causal attention (streaming softmax, denominator
via ones-column in V), then a head-split AllToAll redistributes outputs so
core p holds all 1024 channels for token blocks [128p, 128p+128) of each
1024-token half; output projection is local per 128-token tile.

v2 schedule (vs baseline):
  - all 8 x chunks prefetched up front, each split across the sync+scalar
    HWDGE queues; per-chunk persistent tiles let attention(qc) start as
    soon as qkv(qc) lands.
  - qkv processed in chunk-pairs with et-ct-outer loops so consecutive
    matmuls share a stationary tile (fewer LDWEIGHTS columns on the PE bus).
  - S^T and exp trimmed to causal columns on diagonal k-tiles.
  - projection bias via vector add against a DMA-broadcast bias tile
    (no bias matmuls).
  - AllToAll split per head-half (2 collectives of 131KB per batch-half):
    staging fires per q-chunk, and the tail's last collective overlaps
    proj of the previous part.
"""

import numpy as np

B, T, C, H, D = 2, 2048, 1024, 16, 64
NCORES = 8
HL = H // NCORES        # heads per core = 2
TOK = B * T
TSL = TOK // NCORES     # 512 output tokens per core
SL = 256                # per-batch token slice per core
P = 128
CT = C // P             # 8 contraction tiles
NQC = T // 512          # 4 q-chunks per batch
SCALE = D ** -0.5

_CACHE = {}


def _build_nc():
    import concourse.bass as bass
    import concourse.mybir as mybir
    from concourse import bacc
    from concourse.tile import TileContext

    f32 = mybir.dt.float32
    bf16 = mybir.dt.bfloat16
    AF = mybir.ActivationFunctionType
    ALU = mybir.AluOpType

    nc = bacc.Bacc(
        "TRN2", target_bir_lowering=False, debug=False, num_devices=NCORES
    )

    xT = nc.dram_tensor("xT", [C, TOK], bf16, kind="ExternalInput")
    wqkvT = nc.dram_tensor("wqkvT", [C, 3 * P], bf16, kind="ExternalInput")
    bqkv = nc.dram_tensor("bqkv", [3 * P], f32, kind="ExternalInput")
    wpT = nc.dram_tensor("wpT", [C, C], bf16, kind="ExternalInput")
    bp = nc.dram_tensor("bp", [C], f32, kind="ExternalInput")
    tri = nc.dram_tensor("tri", [P, P], bf16, kind="ExternalInput")
    ident = nc.dram_tensor("ident", [P, P], bf16, kind="ExternalInput")
    y = nc.dram_tensor("y", [TSL, C], f32, kind="ExternalOutput")

    with TileContext(nc, num_cores=NCORES) as tc:
        from contextlib import ExitStack

        with ExitStack() as ctx:
            const = ctx.enter_context(tc.tile_pool(name="const", bufs=1))
            persist = ctx.enter_context(tc.tile_pool(name="persist", bufs=1))
            dram = ctx.enter_context(tc.tile_pool(name="dram", bufs=1, space="DRAM"))

            # ---- constants + qkv weights (gpsimd queue: early, off the
            # x-load queues)
            tri_sb = const.tile([P, P], bf16)
            id_sb = const.tile([P, P], bf16)
            bq_sb = const.tile([P, 3], f32)
            bp_bc = const.tile([P, C], f32)
            w_sb = const.tile([P, CT, 3 * P], bf16)
            wp_sb = const.tile([P, CT, C], bf16)
            nc.gpsimd.dma_start(tri_sb[:], tri[:])
            nc.gpsimd.dma_start(id_sb[:], ident[:])
            nc.gpsimd.dma_start(bq_sb[:], bqkv.rearrange("(et p) -> p et", p=P))

            # warmup collective: absorbs the one-time cc rendezvous cost
            # while the PE is still waiting on the first x chunk
            cc_wu_in = dram.tile([NCORES, 64], bf16, name="ccwi")
            cc_wu_out = dram.tile([NCORES, 64], bf16, name="ccwo")
            nc.gpsimd.collective_compute(
                "AllToAll",
                mybir.AluOpType.bypass,
                replica_groups=[list(range(NCORES))],
                ins=[cc_wu_in.opt()],
                outs=[cc_wu_out.opt()],
            )

            # ---- x chunks: all prefetched, halves split across queues;
            # qkv weights split across both HWDGE queues ahead of x
            wsrc = wqkvT.rearrange("(ct p) e -> p ct e", p=P)
            nc.sync.dma_start(w_sb[:, 0:4, :], wsrc[:, 0:4, :])
            nc.scalar.dma_start(w_sb[:, 4:8, :], wsrc[:, 4:8, :])
            xc = [[persist.tile([P, CT, 512], bf16, name=f"x{b}{c}")
                   for c in range(NQC)] for b in range(B)]
            for b in range(B):
                for c in range(NQC):
                    t0 = b * T + c * 512
                    src = xT[:, t0:t0 + 512].rearrange("(ct p) t -> p ct t", p=P)
                    nc.sync.dma_start(xc[b][c][:, 0:4, :], src[:, 0:4, :])
                    nc.scalar.dma_start(xc[b][c][:, 4:8, :], src[:, 4:8, :])
            # late-need consts
            nc.gpsimd.dma_start(
                bp_bc[:],
                bp.rearrange("(o c) -> o c", o=1).to_broadcast((P, C)),
            )
            nc.gpsimd.dma_start(
                wp_sb[:], wpT.rearrange("(ct p) f -> p ct f", p=P))

            # ---- per-chunk persistent activations
            qTc = [[persist.tile([P, 512], bf16, name=f"q{b}{c}")
                    for c in range(NQC)] for b in range(B)]
            kTc = [[persist.tile([P, 512], bf16, name=f"k{b}{c}")
                    for c in range(NQC)] for b in range(B)]
            # V with ones column: [128 tok, kt, 2*65]
            vaug = [[persist.tile([P, 4, 2 * 65], bf16, name=f"va{b}{c}")
                     for c in range(NQC)] for b in range(B)]
            # normalized A^T (+denominator row) per (b, qc, h)
            anf = [[[persist.tile([65, 512], f32, name=f"an{b}{c}{h}")
                     for h in range(HL)] for c in range(NQC)] for b in range(B)]
            # a2a buffers per (b, part): rows (j, ee) = (dest core, payload)
            # payload rows 0-127 = my 128 channels UNNORMALIZED, rows
            # 128-129 = the two heads' softmax reciprocals for those tokens
            a2a_in = [[dram.tile([NCORES * 130, P], bf16, name=f"ai{b}{pr}")
                       for pr in range(2)] for b in range(B)]
            a2a_out = [[dram.tile([NCORES * 130, P], bf16, name=f"ao{b}{pr}")
                        for pr in range(2)] for b in range(B)]

            pools = [
                tc.tile_pool(name="sps", bufs=2, space="PSUM"),   # 2x2 banks
                tc.tile_pool(name="ops", bufs=2, space="PSUM"),   # 2x1 bank
                tc.tile_pool(name="mm", bufs=2, space="PSUM"),    # 2x1 bank
                tc.tile_pool(name="pT", bufs=2),
                tc.tile_pool(name="vt", bufs=2),
                tc.tile_pool(name="rp", bufs=2),
                tc.tile_pool(name="afull", bufs=2),
                tc.tile_pool(name="afr", bufs=4),
                tc.tile_pool(name="ysb", bufs=2),
            ]
            sps, ops, mm, ppool, vtpool, rppool, apool, rpool, ypool = (
                ctx.enter_context(p) for p in pools)

            def qkv_chunk(b, c):
                """qkv^T for one 512-token chunk."""
                vt = None
                for et in range(3):
                    ps = mm.tile([P, 512], f32, tag="mm")
                    for ct in range(CT):
                        nc.tensor.matmul(
                            ps[:],
                            lhsT=w_sb[:, ct, et * P:(et + 1) * P],
                            rhs=xc[b][c][:, ct, :],
                            start=(ct == 0),
                            stop=(ct == CT - 1),
                        )
                    if et == 0:
                        dst = qTc[b][c][:]
                    elif et == 1:
                        dst = kTc[b][c][:]
                    else:
                        vt = vtpool.tile([P, 512], bf16, tag="vt")
                        dst = vt[:]
                    nc.vector.tensor_scalar_add(
                        dst, ps[:], bq_sb[:, et:et + 1],
                    )
                # V^T -> V (PE transpose) + vaug fill
                nc.vector.memset(vaug[b][c][:, :, 64::65], 1.0)
                for kt4 in range(4):
                    tp = mm.tile([P, P], bf16, tag="mm")
                    nc.tensor.transpose(
                        tp[:], vt[:, kt4 * P:(kt4 + 1) * P], id_sb[:],
                    )
                    nc.vector.tensor_copy(
                        vaug[b][c][:, kt4, 0:2 * 65]
                        .rearrange("p (h e) -> p h e", h=2)[:, :, 0:64],
                        tp.rearrange("p (h e) -> p h e", h=2),
                    )

            def attention_qc(b, qc):
                q0 = qc * 512
                nk = 4 * qc + 4
                o_t = [ops.tile([65, 512], f32, tag="o", name=f"ot{hh}")
                       for hh in range(HL)]

                def s_mm(ki):
                    off = ki * P - q0
                    lo = max(0, off)
                    sp = sps.tile([P, HL, 512], f32, tag="s")
                    for h in range(HL):
                        hp = slice(64 * h, 64 * h + 64)
                        nc.tensor.matmul(
                            sp[:, h, lo:512],
                            lhsT=kTc[b][ki // 4][hp, (ki % 4) * P:(ki % 4 + 1) * P],
                            rhs=qTc[b][qc][hp, lo:512],
                            start=True,
                            stop=True,
                        )
                    return sp

                sp = s_mm(0)
                for ki in range(nk):
                    off = ki * P - q0
                    lo = max(0, off)
                    pt = ppool.tile([P, HL, 512], bf16, tag="p")
                    nc.scalar.activation(
                        pt[:, :, lo:512], sp[:, :, lo:512], AF.Exp, scale=SCALE,
                    )
                    # software pipeline: issue S(ki+1) before AV(ki) so the
                    # in-order PE queue never waits on exp latency
                    if ki + 1 < nk:
                        sp = s_mm(ki + 1)
                    if off >= 0:
                        for h in range(HL):
                            nc.vector.tensor_tensor(
                                pt[:, h, off:off + P],
                                pt[:, h, off:off + P],
                                tri_sb[:],
                                ALU.mult,
                            )
                    for h in range(HL):
                        nc.tensor.matmul(
                            o_t[h][:, lo:512],
                            lhsT=vaug[b][ki // 4][:, ki % 4, h * 65:h * 65 + 65],
                            rhs=pt[:, h, lo:512],
                            start=(ki == 0),
                            stop=(ki == nk - 1),
                        )
                # epilogue: stash unnormalized A^T + its reciprocal row and
                # stage both into the (b, part) a2a payload
                part, j0 = qc // 2, 4 * (qc % 2)
                a2a_v = a2a_in[b][part].rearrange("(j ee) t -> ee j t", j=NCORES)
                for h in range(HL):
                    nc.vector.tensor_copy(anf[b][qc][h][:], o_t[h][:])
                for h in range(HL):
                    at = anf[b][qc][h]
                    nc.gpsimd.dma_start(
                        a2a_v[64 * h:64 * h + 64, j0:j0 + 4, :],
                        at[0:64, :].rearrange("e (j t) -> e j t", j=4),
                    )
                    dpk = rppool.tile([8, 64], f32, tag="dpk")
                    rpk = rppool.tile([8, 64], f32, tag="rpk")
                    rsc = rppool.tile([8, 64], f32, tag="rsc")
                    nc.sync.dma_start(dpk[:], at[64:65, :])
                    nc.vector.reciprocal_approx_accurate(rpk[:], dpk[:], rsc[:])
                    nc.gpsimd.dma_start(
                        a2a_v[128 + h:129 + h, j0:j0 + 4, :], rpk[:],
                    )

            def a2a_fire(b, part):
                nc.gpsimd.collective_compute(
                    "AllToAll",
                    mybir.AluOpType.bypass,
                    replica_groups=[list(range(NCORES))],
                    ins=[a2a_in[b][part].opt()],
                    outs=[a2a_out[b][part].opt()],
                )

            def proj_part(b, part):
                # A rows and reciprocal rows from the collective
                af = apool.tile([P, NCORES, P], bf16, tag="af")
                afr = [rpool.tile([1, NCORES, P], bf16, tag="afr",
                                  name=f"afr{b}{part}{hh}")
                       for hh in range(HL)]
                src = a2a_out[b][part].rearrange("(i e) t -> e i t", e=130)
                nc.scalar.dma_start(af[:], src[0:128])
                for h in range(HL):
                    nc.scalar.dma_start(afr[h][:], src[128 + h:129 + h])
                # normalize: broadcast recip over 64 partitions via PE, then
                # scale af in SBUF
                for h in range(HL):
                    for ih in range(2):
                        rb2 = mm.tile([64, 512], f32, tag="mm")
                        nc.tensor.matmul(
                            rb2[:],
                            lhsT=tri_sb[0:1, 0:64],
                            rhs=afr[h][0:1, 4 * ih:4 * ih + 4, :],
                            start=True,
                            stop=True,
                        )
                        nc.vector.tensor_tensor(
                            af[64 * h:64 * h + 64, 4 * ih:4 * ih + 4, :],
                            af[64 * h:64 * h + 64, 4 * ih:4 * ih + 4, :],
                            rb2.rearrange("p (i t) -> p i t", i=4),
                            ALU.mult,
                        )
                for fc in range(2):
                    ps = mm.tile([P, 512], f32, tag="mm")
                    for i in range(NCORES):
                        nc.tensor.matmul(
                            ps[:],
                            lhsT=af[:, i, :],
                            rhs=wp_sb[:, i, fc * 512:(fc + 1) * 512],
                            start=(i == 0),
                            stop=(i == NCORES - 1),
                        )
                    ysb = ypool.tile([P, 512], f32, tag="ysb")
                    nc.vector.tensor_tensor(
                        ysb[:], ps[:], bp_bc[:, fc * 512:(fc + 1) * 512],
                        ALU.add,
                    )
                    r0 = b * SL + part * P
                    eng = nc.sync if fc == 0 else nc.scalar
                    eng.dma_start(
                        y[r0:r0 + P, fc * 512:(fc + 1) * 512], ysb[:],
                    )

            # ---------------- schedule ----------------
            qkv_chunk(0, 0)
            attention_qc(0, 0)
            qkv_chunk(0, 1)
            attention_qc(0, 1)
            a2a_fire(0, 0)
            qkv_chunk(0, 2)
            attention_qc(0, 2)
            qkv_chunk(0, 3)
            attention_qc(0, 3)
            a2a_fire(0, 1)
            qkv_chunk(1, 0)
            attention_qc(1, 0)
            proj_part(0, 0)
            qkv_chunk(1, 1)
            attention_qc(1, 1)
            a2a_fire(1, 0)
            qkv_chunk(1, 2)
            attention_qc(1, 2)
            proj_part(0, 1)
            qkv_chunk(1, 3)
            attention_qc(1, 3)
            proj_part(1, 0)
            a2a_fire(1, 1)
            proj_part(1, 1)
    nc.compile()
    return nc


def _prep_inputs(x, W_qkv, b_qkv, W_proj, b_proj):
    x = np.asarray(x, dtype=np.float32)
    W_qkv = np.asarray(W_qkv, dtype=np.float32)
    b_qkv = np.asarray(b_qkv, dtype=np.float32)
    W_proj = np.asarray(W_proj, dtype=np.float32)
    b_proj = np.asarray(b_proj, dtype=np.float32)

    import ml_dtypes
    bf = ml_dtypes.bfloat16
    xT = np.ascontiguousarray(x.reshape(TOK, C).T).astype(bf)
    wpT = np.ascontiguousarray(W_proj.T).astype(bf)
    tri = np.triu(np.ones((P, P), dtype=np.float32)).astype(bf)
    ident = np.eye(P, dtype=np.float32).astype(bf)

    in_maps = []
    for p in range(NCORES):
        rows = np.r_[128 * p:128 * p + 128,
                     C + 128 * p:C + 128 * p + 128,
                     2 * C + 128 * p:2 * C + 128 * p + 128]
        wslice = W_qkv[rows]
        bslice = np.ascontiguousarray(b_qkv[rows])
        in_maps.append({
            "xT": xT,
            "wqkvT": np.ascontiguousarray(wslice.T).astype(bf),
            "bqkv": bslice,
            "wpT": wpT,
            "bp": b_proj,
            "tri": tri,
            "ident": ident,
        })
    return in_maps


def kernel(x, W_qkv, b_qkv, W_proj, b_proj, _trace=False):
    from concourse import bass_utils

    if "nc" not in _CACHE:
        _CACHE["nc"] = _build_nc()
    nc = _CACHE["nc"]
    in_maps = _prep_inputs(x, W_qkv, b_qkv, W_proj, b_proj)
    res = bass_utils.run_bass_kernel_spmd(
        nc, in_maps, core_ids=list(range(NCORES)), trace=_trace,
    )
    _CACHE["last_result"] = res
    yfull = np.empty((B, T, C), dtype=np.float32)
    for p, rmap in enumerate(res.results):
        yp = rmap["y"]
        for b in range(B):
            for part in range(2):
                g0 = part * 1024 + 128 * p
                r0 = b * SL + part * P
                yfull[b, g0:g0 + P] = yp[r0:r0 + P]
    return yfull
